# revision 29
# baseline (speedup 1.0000x reference)
import numpy as np

# nn_CorrLayerDownsample: J=3, L=8, M=N=256, NB=2, 7 shift positions.
# out[(j1,j2)][b, l1, l2, s] = sum_p x1[b,l1,p+s] * up(x2)[b,l2,p]  (circular),
# where up() is the real part of the spectral (Fourier zero-pad) upsample of
# the coarser scale j2 onto the j1 grid.
#
# Device scheme: groups sharing j1 reuse the same shifted-x1 operand.
# Per 128-pixel chunk: one matmul with stationary lhsT = [128 px, 128 cols =
# (2 batch x 8 shift-slot x 8 l1)] (shift slot 7 zero-padded; 128 contiguous
# bf16 weight cols enable FWL) and moving rhs = [128 px, NW cols = (groups x
# 2 batch x 8 l2)], PSUM-accumulated over the core's chunks. Each core owns
# 1/8 of the image rows; host sums the 8 per-core partials and extracts the
# valid (b == b') blocks. A and W are packed into one DRAM tensor per stage
# piece, so each piece is one large fully-contiguous DMA (~378 GB/s measured,
# at the per-core HBM cap; the kernel is DMA-bound: 3.69 MB/core vs ~4 us of
# PE work). Primary path is raw bass (manual semaphores; one sem per DMA --
# sem increments to a shared sem serialize at ~26 ns each); fallback is a
# Tile-context build, then pure numpy. This stack's walrus build rejects >1
# sync-wait on DMA instructions and only a few on CTRL: the raw path keeps
# every instruction at <=1 wait, and the Tile path needs _patch_tile_drain
# to split the kernel-tail drain's waits across SP nops.

J, L, M, N, NB = 3, 8, 256, 256, 2
SHIFTS = [(0, 0), (0, 1), (0, 2), (1, 0), (1, 1), (2, 0), (-1, 1)]
NSHIFT = len(SHIFTS)
NCORES = 8
NCHS = [64, 16, 4]          # chunks per core: (h*w/8)/128 for h=256,128,64
NWS = [48, 32, 16]          # (num groups) * 2 batches * 8 l2
MA = 128                    # stationary cols: 2b x 8 shift-slots x 8 l1 (slot 7 zero-padded: FWL needs 128; A/B-measured ~2us faster than m=112)
DSPLIT = [2, 1, 1]          # input DMA pieces per stage (total 4 + 3 out)


def _upsample(x, Mn, Nn):
    # real-part spectral zero-pad upsample, matching the reference's
    # fftshift/pad/ifftshift convention
    m, n = x.shape[-2], x.shape[-1]
    if (m, n) == (Mn, Nn):
        return np.asarray(x, np.float64)
    xh = np.fft.fft2(x)
    xs = np.fft.fftshift(xh, axes=(-2, -1))
    ph, pw = (Mn - m) // 2, (Nn - n) // 2
    pad = [(0, 0)] * (x.ndim - 2) + [(ph, ph), (pw, pw)]
    xp = np.pad(xs, pad)
    xh2 = np.fft.ifftshift(xp, axes=(-2, -1)) * ((Mn * Nn) / (m * n))
    return np.fft.ifft2(xh2).real


def _numpy_compute(xs):
    # exact fallback: same math via numpy FFTs (mirrors reference; verified
    # 1.3e-7 vs the jax reference on cpu)
    la1 = np.repeat(np.arange(L), L)
    la2 = np.tile(np.arange(L), L)
    outs = []
    hats = [np.fft.fft2(x.astype(np.complex128)) for x in xs]
    for j1, j2 in [(0, 0), (0, 1), (0, 2), (1, 1), (1, 2), (2, 2)]:
        h, w = M >> j1, N >> j1
        h1 = hats[j1][:, la1]
        h2 = hats[j2][:, la2]
        if j2 > j1:
            m, n = M >> j2, N >> j2
            xsft = np.fft.fftshift(h2, axes=(-2, -1))
            ph, pw = (h - m) // 2, (w - n) // 2
            xp = np.pad(xsft, [(0, 0), (0, 0), (ph, ph), (pw, pw)])
            h2 = np.fft.ifftshift(xp, axes=(-2, -1)) * ((h * w) / (m * n))
        corr = np.fft.ifft2(h1 * np.conj(h2)).real
        flat = corr.reshape(corr.shape[0], corr.shape[1], h * w)
        uidx = np.array(sorted(((dx % h) * w + (dy % w)) for dx, dy in SHIFTS))
        outs.append(flat[:, :, uidx])
    return np.concatenate(outs, axis=1).astype(np.float32)


def _pack_stage(x1, fields, nch):
    # x1: [2, 8, h, w] f32. fields: list of [2, 8, h, w] (the x2 side of each
    # group at this j1). Returns per-core merged [128, nch, 128 + G*16] bf16
    # in pixel-on-partition layout: cols [0:128] = shifted x1 (b, s, l1),
    # cols [128:] = x2 fields (G, b, l2).
    import ml_dtypes

    shifted = np.stack(
        [np.roll(x1, (-dx, -dy), axis=(2, 3)) for dx, dy in SHIFTS]
    )  # [7, b, l1, h, w]
    A = shifted.astype(ml_dtypes.bfloat16).reshape(NSHIFT, NB, L, NCORES, nch, 128)
    A = A.transpose(3, 5, 4, 1, 0, 2)  # [k, p, c, b, s, l1]
    Wf = np.stack(fields).astype(ml_dtypes.bfloat16)  # [G, b, l2, h, w]
    G = Wf.shape[0]
    Wp = Wf.reshape(G, NB, L, NCORES, nch, 128).transpose(3, 5, 4, 0, 1, 2)
    out = np.zeros((NCORES, 128, nch, MA + G * 16), ml_dtypes.bfloat16)
    out[..., :MA].reshape(NCORES, 128, nch, NB, 8, L)[:, :, :, :, :NSHIFT, :] = A
    out[..., MA:] = Wp.reshape(NCORES, 128, nch, G * 16)
    return out


def _patch_tile_drain(max_waits=1):
    # This stack's walrus build rejects CTRL instructions carrying more than
    # a few sync waits ("Too many sync wait commands") and Tile's kernel-tail
    # drain collects one wait per sem lane. Split the drain's waits across
    # extra SP nops, each carrying <= max_waits.
    import concourse.mybir as mybir
    from concourse.tile import TileContext
    from concourse.vector_clock import ScopedClock

    if getattr(TileContext, "_drain_split_patched", False):
        return

    def _drain_and_barrier(self, tick_clock, wait_clock):
        drain_inst = self.nc.sync.drain()
        wait_clock.add_sem_waits(
            drain_inst.ins, ScopedClock({None: tick_clock.global_clock})
        )
        si = drain_inst.ins.sync_info
        waits = list(si.on_wait) if si is not None else []
        if len(waits) > max_waits:
            drain_inst.ins.sync_info = mybir.SyncInfo(
                on_wait=waits[:max_waits], on_update=list(si.on_update)
            )
            rest = waits[max_waits:]
            for i in range(0, len(rest), max_waits):
                nop = self.nc.sync.nop(nofuse=True, hint="drain_wait_split")
                nop.ins.sync_info = mybir.SyncInfo(
                    on_wait=rest[i : i + max_waits], on_update=[]
                )

        self.nc.all_engine_barrier()
        assert self.sems is not None
        popped = self.nc._tile_sem_poison_stack.pop()
        assert popped is self._sem_poison
        self.nc.clear_and_free_semaphores(list(self.sems.allocated().values()))
        self.nc.all_engine_barrier()

    TileContext._drain_and_barrier = _drain_and_barrier
    TileContext._drain_split_patched = True


def _build_nc():
    import concourse.bass as bass
    import concourse.mybir as mybir
    from concourse.tile import TileContext

    _patch_tile_drain()

    bf16, f32 = mybir.dt.bfloat16, mybir.dt.float32
    nc = bass.Bass()
    s_d = []
    for st in range(3):
        nch, nw, nsp = NCHS[st], NWS[st], DSPLIT[st]
        cs = nch // nsp
        s_d.append(
            [
                nc.dram_tensor(
                    f"s{st}_{i}", [128, cs, MA + nw], bf16, kind="ExternalInput"
                )
                for i in range(nsp)
            ]
        )
    # single output tensor/DMA keeps the total DMA-lane count low: the
    # kernel-tail drain waits on every sem lane and supports at most 8 waits
    o_d = nc.dram_tensor("o", [MA, sum(NWS)], f32, kind="ExternalOutput")

    with TileContext(nc) as tc:
        with (
            tc.tile_pool(name="sb", bufs=1) as pool,
            tc.tile_pool(name="ps", bufs=1, space="PSUM") as pp,
            tc.tile_pool(name="ob", bufs=1) as op,
        ):
            tiles = []
            for st in range(3):
                nch, nw, nsp = NCHS[st], NWS[st], DSPLIT[st]
                cs = nch // nsp
                tp = []
                for i in range(nsp):
                    t = pool.tile([128, cs, MA + nw], bf16, tag=f"s{st}_{i}")
                    nc.sync.dma_start(t[:], s_d[st][i][:])
                    tp.append(t)
                tiles.append(tp)
            ot = op.tile([MA, sum(NWS)], f32, tag="ot")
            off = 0
            for st in range(3):
                nch, nw, nsp = NCHS[st], NWS[st], DSPLIT[st]
                cs = nch // nsp
                ps = pp.tile([MA, nw], f32, tag=f"p{st}")
                for c in range(nch):
                    t = tiles[st][c // cs]
                    nc.tensor.matmul(
                        ps[:],
                        t[:, c % cs, 0:MA],
                        t[:, c % cs, MA : MA + nw],
                        start=(c == 0),
                        stop=(c == nch - 1),
                    )
                nc.vector.tensor_copy(ot[:, off : off + nw], ps[:])
                off += nw
            nc.scalar.dma_start(o_d[:], ot[:])
    return nc


def _build_nc_raw(skip_preamble=True):
    # Raw-bass variant (no TileContext): manual semaphores, input DMAs issued
    # first on SP, PE waits per piece, DVE copies PSUM->SBUF, ACT does the
    # single output DMA and waits for its completion. Avoids Tile's scheduling
    # prologue, drain-wait storm, and sem-clear storm.
    import contextlib

    import concourse.bass as bass
    import concourse.mybir as mybir

    bf16, f32 = mybir.dt.bfloat16, mybir.dt.float32
    if skip_preamble:
        # Trim the constructor-emitted prologue: (a) the per-engine register
        # preamble (unused here: no register-offset APs), (b) the
        # all-engine barrier after the const-tile memsets -- in this raw
        # kernel every cross-engine dependency is semaphore-mediated, and
        # the init sem-clears are already ordered by _nrt_pseudo_barrier,
        # so nothing needs that barrier.
        # Also skip the init-time kernel-sem-range clears + their pseudo
        # barrier: sems are zero on a fresh NEFF load and this kernel's
        # gpsimd tail re-zeros every sem it touches, so re-execution is
        # clean without them.
        orig_preamble = bass.BassEngine.preamble
        orig_barrier = bass.Bass.all_engine_barrier
        orig_pseudo = bass.Bass._nrt_pseudo_barrier
        orig_reset = bass.BassGpSimd.dma_reset
        orig_clear = bass.BassGpSimd.sem_clear
        bass.BassEngine.preamble = lambda self: None
        bass.Bass.all_engine_barrier = lambda self, **kw: None
        bass.Bass._nrt_pseudo_barrier = lambda self: None
        bass.BassGpSimd.dma_reset = lambda self, *a, **kw: None
        bass.BassGpSimd.sem_clear = lambda self, *a, **kw: None
        try:
            nc = bass.Bass()
        finally:
            bass.BassEngine.preamble = orig_preamble
            bass.Bass.all_engine_barrier = orig_barrier
            bass.Bass._nrt_pseudo_barrier = orig_pseudo
            bass.BassGpSimd.dma_reset = orig_reset
            bass.BassGpSimd.sem_clear = orig_clear
    else:
        nc = bass.Bass()
    s_d = []
    for st in range(3):
        nch, nw, nsp = NCHS[st], NWS[st], DSPLIT[st]
        cs = nch // nsp
        s_d.append(
            [
                nc.dram_tensor(
                    f"s{st}_{i}", [128, cs, MA + nw], bf16, kind="ExternalInput"
                )
                for i in range(nsp)
            ]
        )
    o_d = nc.dram_tensor("o", [MA, sum(NWS)], f32, kind="ExternalOutput")

    with contextlib.ExitStack() as ctx:
        tiles = []
        pieces = []  # (st, piece_idx, tile)
        for st in range(3):
            nch, nw, nsp = NCHS[st], NWS[st], DSPLIT[st]
            cs = nch // nsp
            tp = []
            for i in range(nsp):
                t = ctx.enter_context(
                    nc.sbuf_tensor(f"t{st}_{i}", [128, cs, MA + nw], bf16)
                )
                tp.append(t)
                pieces.append((st, i, t))
            tiles.append(tp)
        pss = [
            ctx.enter_context(nc.psum_tensor(f"ps{st}", [MA, NWS[st]], f32))
            for st in range(3)
        ]
        ot = ctx.enter_context(nc.sbuf_tensor("ot", [MA, sum(NWS)], f32))
        dsems = [
            ctx.enter_context(nc.semaphore(f"dsem{i}")) for i in range(len(pieces))
        ]
        psem = ctx.enter_context(nc.semaphore("psem"))
        vsem = ctx.enter_context(nc.semaphore("vsem"))
        osem = ctx.enter_context(nc.semaphore("osem"))
        block = ctx.enter_context(nc.Block(no_gpsimd_drain=True))

        @block.sync
        def _(sync):
            for pi, (st, i, t) in enumerate(pieces):
                sync.dma_start(t[:], s_d[st][i][:]).then_inc(dsems[pi], 16)

        @block.scalar
        def _(sc):
            # single output DMA on the ACT ring (q10), separate from the input
            # queue so it never sits behind straggler input packets
            sc.wait_ge(vsem, 3)
            sc.dma_start(o_d[:], ot[:]).then_inc(osem, 16)
            sc.wait_ge(osem, 16)

        @block.tensor
        def _(pe):
            ndone = 0
            for st in range(3):
                nch, nw, nsp = NCHS[st], NWS[st], DSPLIT[st]
                cs = nch // nsp
                for i in range(nsp):
                    pe.wait_ge(dsems[ndone], 16)
                    ndone += 1
                    for cc in range(cs):
                        c = i * cs + cc
                        mm = nc.tensor.matmul(
                            pss[st][:],
                            tiles[st][i][:, cc, 0:MA],
                            tiles[st][i][:, cc, MA : MA + nw],
                            start=(c == 0),
                            stop=(c == nch - 1),
                        )
                        if c == nch - 1:
                            mm.then_inc(psem, 1)

        @block.vector
        def _(v):
            off = 0
            for st in range(3):
                v.wait_ge(psem, st + 1)
                nc.vector.tensor_copy(
                    ot[:, off : off + NWS[st]], pss[st][:]
                ).then_inc(vsem, 1)
                off += NWS[st]

        @block.gpsimd
        def _(gp):
            # reset sems so a cached re-execution of this NEFF starts clean
            for s in dsems:
                gp.wait_ge(s, 16)
            gp.wait_ge(osem, 16)
            for s in dsems + [psem, vsem, osem]:
                gp.sem_clear(s)

    return nc


def _prepare_in_maps(xs):
    u1f = _upsample(xs[1], 256, 256)
    u2f = _upsample(xs[2], 256, 256)
    u2m = _upsample(xs[2], 128, 128)
    packs = [
        _pack_stage(xs[0], [xs[0], u1f, u2f], NCHS[0]),
        _pack_stage(xs[1], [xs[1], u2m], NCHS[1]),
        _pack_stage(xs[2], [xs[2]], NCHS[2]),
    ]
    in_maps = []
    for k in range(NCORES):
        m = {}
        for st in range(3):
            nsp = DSPLIT[st]
            cs = NCHS[st] // nsp
            for i in range(nsp):
                m[f"s{st}_{i}"] = np.ascontiguousarray(
                    packs[st][k][:, i * cs : (i + 1) * cs]
                )
        in_maps.append(m)
    return in_maps


def _reduce(results):
    # per-stage: sum partials over cores, reshape m=(b,s,l1) n=(G,b2,l2),
    # keep b==b2 and s<7, emit groups in global order.
    out = np.zeros((NB, 6 * L * L, NSHIFT), np.float32)
    goff = 0
    noff = 0
    for st, ngrp in zip(range(3), (3, 2, 1)):
        O = np.zeros((MA, NWS[st]), np.float64)
        for r in results:
            O += r["o"][:, noff : noff + NWS[st]].astype(np.float64)
        noff += NWS[st]
        O = O.reshape(NB, 8, L, ngrp, NB, L)  # [b, s-slot, l1, g, b2, l2]
        for g in range(ngrp):
            for b in range(NB):
                blk = O[b, :NSHIFT, :, g, b, :]  # [s, l1, l2]
                out[b, (goff + g) * 64 : (goff + g + 1) * 64, :] = (
                    blk.transpose(1, 2, 0).reshape(L * L, NSHIFT)
                )
        goff += ngrp
    return out


def _run_bass(in_maps, trace=False, raw=True):
    from concourse.bass_utils import run_bass_kernel_spmd

    nc = _build_nc_raw() if raw else _build_nc()
    return run_bass_kernel_spmd(nc, in_maps, list(range(NCORES)), trace=trace)


_last_used_bass = False


def kernel(xpsi_0, xpsi_1, xpsi_2):
    global _last_used_bass
    _last_used_bass = False
    xs = [
        np.asarray(xpsi_0, np.float32),
        np.asarray(xpsi_1, np.float32),
        np.asarray(xpsi_2, np.float32),
    ]
    try:
        import signal

        def _abort(signum, frame):
            raise TimeoutError("bass path timed out")

        old = signal.signal(signal.SIGALRM, _abort)
        signal.alarm(1500)
        try:
            in_maps = _prepare_in_maps(xs)
            try:
                res = _run_bass(in_maps, raw=True)
            except Exception:
                res = _run_bass(in_maps, raw=False)
        finally:
            signal.alarm(0)
            signal.signal(signal.SIGALRM, old)
        out = _reduce(res.results)
        _last_used_bass = True
        return out
    except Exception:
        return _numpy_compute(xs)
